# revision 42
# baseline (speedup 1.0000x reference)
"""Multi-head causal attention (B=4, S=2048, D=1024, H=16) on 8 TRN2 cores.

Sharding: core = (batch, head-group): 4 batches x 2 groups of 8 heads.
Every core runs an identical program (uniform causal structure -> valid SPMD):
  - Q/K/V projections for its 8 heads over its batch's full 2048 rows
    (Q,K produced transposed [Dout, S]; V natural [S, Dout] + ones column)
  - causal attention per head-pair: scores_T = K_h @ Q_h^T via row-packed
    K=64 matmuls (tile_position); ACT exp straight from PSUM; multiplicative
    0/1 triangle mask on the diagonal 128-strip post-exp (DVE); attnV with
    M=65 (65th row accumulates the softmax denominator); reciprocal +
    gpsimd partition-broadcast; normalize into outT. The (pair, keytile)
    stream is software-pipelined (scores i+1 before attnV i) and projection/
    output-projection work for neighboring blocks is woven in as PE filler.
  - partial output projection y_part = outT^T @ Wo^T (contraction over this
    group's 512 channels)
Host: y[b] = y_part[b,g0] + y_part[b,g1] + bo.

All matmuls run as float32r (fp32 with 11-bit mantissa, ~tf32): inputs are
pre-rounded on host, intermediates rounded by the producing DVE/ACT op.
"""
import sys

for _p in ("/opt/trn_rl_repo", "/root/.axon_site/_ro/trn_rl_repo"):
    if _p not in sys.path:
        sys.path.append(_p)

import numpy as np
from contextlib import ExitStack

B, S, D, H = 4, 2048, 1024, 16
DK = D // H          # 64
HG = H // 2          # 8 heads per core
DG = HG * DK         # 512 channels per core
P = 128
NQB = S // 512       # 4 query blocks of 512
NKT = S // P         # 16 key tiles of 128
MASK_VAL = -1e5

_cache = {}


def _to_f32r(x):
    b = np.ascontiguousarray(x, dtype=np.float32).view(np.uint32)
    r = np.bitwise_and(b + np.uint32(0x800), np.uint32(0xFFFFF000))
    return r.view(np.float32)


def _build():
    import concourse.tile as tile
    from concourse import bacc, mybir

    f32 = mybir.dt.float32
    f32r = mybir.dt.float32r
    Exp = mybir.ActivationFunctionType.Exp

    nc = bacc.Bacc("TRN2", target_bir_lowering=False, debug=False,
                   enable_asserts=False, num_devices=8)

    xq = nc.dram_tensor("xq", [D, S], f32r, kind="ExternalInput").ap()
    xk = nc.dram_tensor("xk", [D, S], f32r, kind="ExternalInput").ap()
    xv = nc.dram_tensor("xv", [D, S], f32r, kind="ExternalInput").ap()
    wq = nc.dram_tensor("wq", [D, DG], f32r, kind="ExternalInput").ap()
    wk = nc.dram_tensor("wk", [D, DG], f32r, kind="ExternalInput").ap()
    wv = nc.dram_tensor("wv", [D, DG], f32r, kind="ExternalInput").ap()
    wo = nc.dram_tensor("wo", [DG, D], f32r, kind="ExternalInput").ap()
    bq = nc.dram_tensor("bq", [P, DG // P], f32, kind="ExternalInput").ap()
    bk = nc.dram_tensor("bk", [P, DG // P], f32, kind="ExternalInput").ap()
    bvr = nc.dram_tensor("bvr", [P, DG], f32, kind="ExternalInput").ap()
    masktri = nc.dram_tensor("masktri", [P, P], f32r, kind="ExternalInput").ap()
    onescol = nc.dram_tensor("onescol", [P, NKT, HG], f32r, kind="ExternalInput").ap()
    y = nc.dram_tensor("y", [S, D], f32, kind="ExternalOutput").ap()

    with tile.TileContext(nc) as tc, ExitStack() as ctx:
        persist = ctx.enter_context(tc.tile_pool(name="persist", bufs=1))
        consts = ctx.enter_context(tc.tile_pool(name="consts", bufs=1))
        xin_pool = ctx.enter_context(tc.tile_pool(name="xin", bufs=8))
        w_pool = ctx.enter_context(tc.tile_pool(name="w", bufs=6))
        wo_pool = ctx.enter_context(tc.tile_pool(name="wop", bufs=2))
        ot_pool = ctx.enter_context(tc.tile_pool(name="otpool", bufs=2))
        sb_small = ctx.enter_context(tc.tile_pool(name="sbs", bufs=2))
        y_pool = ctx.enter_context(tc.tile_pool(name="ysb", bufs=2))
        exp_pool = ctx.enter_context(tc.tile_pool(name="expp", bufs=3))
        # (bufs=4 exp made no difference; 3 leaves SBUF slack)

        # persistent intermediates, subtiled for fine-grained scheduling
        KTs = [[persist.tile([P, 512], f32r, tag=f"KT{m}_{qc}",
                             name=f"KT{m}_{qc}")
                for qc in range(4)] for m in range(4)]
        Vgs = [persist.tile([P, HG, DK + 1], f32r, tag=f"Vg{kt}",
                            name=f"Vg{kt}")
               for kt in range(NKT)]
        qt_pool = ctx.enter_context(tc.tile_pool(name="qtp", bufs=2))

        mask_t = consts.tile([P, P], f32r, tag="mask")
        bq_t = consts.tile([P, DG // P], f32, tag="bq")
        bk_t = consts.tile([P, DG // P], f32, tag="bk")
        bvr_t = consts.tile([P, DG], f32, tag="bvr")

        def load_xin(src, qb):
            xts = []
            for j in range(D // P):
                xt = xin_pool.tile([P, 512], f32r, tag="xin")
                nc.sync.dma_start(
                    xt[:], src[j * P:(j + 1) * P, qb * 512:(qb + 1) * 512])
                xts.append(xt)
            return xts

        def load_w_halves(wsrc):
            whs = []
            for h2 in range(2):
                wh = w_pool.tile([P, D // P, DG // 2], f32r, tag="wh")
                nc.sync.dma_start(
                    wh[:], wsrc[:, h2 * (DG // 2):(h2 + 1) * (DG // 2)]
                    .rearrange("(o p) m -> p o m", p=P))
                whs.append(wh)
            return whs

        sc_ps = ctx.enter_context(tc.tile_pool(name="scps", bufs=2, space="PSUM"))
        proj_ps = sc_ps
        oa_ps = ctx.enter_context(tc.tile_pool(name="oaps", bufs=1, space="PSUM"))

        # prefetch the exp table-set during the projection phase
        warm = consts.tile([1, 4], f32, tag="warm")
        nc.scalar.activation(warm[:], mask_t[0:1, 0:4], Exp)

        def proj_qk_m(whs, xts, bias_t, dst_tile, m):
            ps2 = proj_ps.tile([P, 2, 512], f32, tag="sc", name="pps")
            ps = ps2[:, 0, :]
            for j in range(D // P):
                nc.tensor.matmul(
                    ps[:], whs[m // 2][:, j, (m % 2) * P:(m % 2 + 1) * P],
                    xts[j][:], start=(j == 0), stop=(j == D // P - 1))
            nc.vector.tensor_scalar_add(
                dst_tile[:], ps[:], bias_t[:, m:m + 1])

        def proj_v_part(whs, xts, qb, mt_l, h2):
            mt = qb * 4 + mt_l
            ps2 = proj_ps.tile([P, 2, 512], f32, tag="sc", name="ppsv")
            ps = ps2[:, 0, :DG // 2]
            for j in range(D // P):
                nc.tensor.matmul(
                    ps[:], xts[j][:, mt_l * P:(mt_l + 1) * P],
                    whs[h2][:, j, :],
                    start=(j == 0), stop=(j == D // P - 1))
            nc.vector.tensor_add(
                Vgs[mt][:, h2 * 4:(h2 + 1) * 4, 0:DK],
                ps[:].rearrange("p (h d) -> p h d", h=4),
                bvr_t[:, h2 * 256:(h2 + 1) * 256]
                .rearrange("p (h d) -> p h d", h=4))

        def yproj_part(qb, OT, qt_l, nb):
            ps2 = sc_ps.tile([P, 2, 512], f32, tag="sc", name="yps")
            ps = ps2[:, 0, :]
            for t in range(DG // P):
                nc.tensor.matmul(
                    ps[:], OT[:, t, qt_l * P:(qt_l + 1) * P],
                    wos[nb][:, t, :],
                    start=(t == 0), stop=(t == DG // P - 1))
            ys = y_pool.tile([P, 512], f32, tag="ys", name="ys")
            nc.vector.tensor_copy(ys[:], ps[:])
            nc.sync.dma_start(
                y[(qb * 4 + qt_l) * P:(qb * 4 + qt_l + 1) * P,
                  nb * 512:(nb + 1) * 512],
                ys[:])

        def attention_qb(qb, QTcur, fillers):
            nfill0 = len(fillers)
            kmax = 4 * (qb + 1)
            OT = ot_pool.tile([P, HG // 2, 512], f32r, tag="OT", name="OT")
            blocks = [(p, kt) for p in range(HG // 2) for kt in range(kmax)]
            oas_by_p = {}
            ex_by_blk = {}

            def emit_scores(p, kt):
                sc = sc_ps.tile([P, 2, 512], f32, tag="sc", name="sc")
                diag = kt >= 4 * qb
                f0 = (kt - 4 * qb) * P if diag else 0
                for hh in (0, 1):
                    nc.tensor.matmul(
                        sc[:, hh, f0:512],
                        KTs[p][kt // 4][hh * DK:(hh + 1) * DK,
                                        (kt % 4) * P:(kt % 4 + 1) * P],
                        QTcur[p][hh * DK:(hh + 1) * DK, f0:512],
                        start=True, stop=True,
                        tile_position=(hh * DK, 0))
                ex = exp_pool.tile([P, 2, 512], f32r, tag="ex", name="ex")
                nc.scalar.activation(ex[:, :, f0:512], sc[:, :, f0:512], Exp)
                if diag:
                    # causal mask, multiplicative post-exp: only the leading
                    # 128-col strip of the valid range is triangular
                    nc.vector.tensor_mul(
                        ex[:, :, f0:f0 + P],
                        ex[:, :, f0:f0 + P],
                        mask_t[:, None, 0:P].to_broadcast((P, 2, P)))
                ex_by_blk[(p, kt)] = ex

            def emit_attnv(p, kt):
                diag = kt >= 4 * qb
                f0 = (kt - 4 * qb) * P if diag else 0
                if kt == 0:
                    t0 = (2 * p) % 4
                    oa0 = oa_ps.tile([DK + 1, 512], f32, tag=f"oa{t0}",
                                     name="oa0")
                    oa1 = oa_ps.tile([DK + 1, 512], f32, tag=f"oa{t0 + 1}",
                                     name="oa1")
                    oas_by_p[p] = (oa0, oa1)
                oas = oas_by_p[p]
                ex = ex_by_blk.pop((p, kt))
                for hh in (0, 1):
                    nc.tensor.matmul(
                        oas[hh][:, f0:512],
                        Vgs[kt][:, 2 * p + hh, :],
                        ex[:, hh, f0:512],
                        start=(kt == 0), stop=(kt == kmax - 1))
                if kt == kmax - 1:
                    for hh in (0, 1):
                        rs = sb_small.tile([1, 512], f32, tag="rs", name="rs")
                        nc.vector.reciprocal(rs[:], oas[hh][DK:DK + 1, :])
                        rb_sb = sb_small.tile([DK, 512], f32, tag="rbsb",
                                              name="rbsb")
                        nc.gpsimd.partition_broadcast(rb_sb[:], rs[:])
                        nc.vector.tensor_mul(
                            OT[hh * DK:(hh + 1) * DK, p, :],
                            oas[hh][0:DK, :], rb_sb[:])
                    # weave filler work at pair boundaries
                    target_done = (nfill0 * (p + 1)) // (HG // 2)
                    while fillers and nfill0 - len(fillers) < target_done:
                        fillers.pop(0)()

            # software pipeline: scores(i+1) issued before attnV(i)
            emit_scores(*blocks[0])
            for i in range(len(blocks)):
                if i + 1 < len(blocks):
                    emit_scores(*blocks[i + 1])
                emit_attnv(*blocks[i])
            for f in fillers:
                f()
            del fillers[:]
            return OT

        def proj_block(qb):
            """Emit projections for block qb; returns (QTcur, filler list)."""
            fillers = []
            xts = load_xin(xk, qb)
            for m in range(DG // P):
                fillers.append(
                    lambda m=m, xts=xts, qb=qb:
                    proj_qk_m(wk_hs, xts, bk_t, KTs[m][qb], m))
            xtsv = load_xin(xv, qb)
            for mt_l in range(4):
                for h2 in range(2):
                    fillers.append(
                        lambda mt_l=mt_l, h2=h2, xtsv=xtsv, qb=qb:
                        proj_v_part(wv_hs, xtsv, qb, mt_l, h2))
            QTcur = []
            for m in range(DG // P):
                qt_t = qt_pool.tile([P, 512], f32r, tag=f"QTm{m}",
                                    name=f"QTm{m}")
                QTcur.append(qt_t)
            xtsq = load_xin(xq, qb)
            for m in range(DG // P):
                fillers.append(
                    lambda m=m, xtsq=xtsq: proj_qk_m(
                        wq_hs, xtsq, bq_t, QTcur[m], m))
            return QTcur, fillers

        # block 0: interleave weight/const loads with their first users
        wk_hs = load_w_halves(wk)
        nc.sync.dma_start(bk_t[:], bk)
        nc.sync.dma_start(bq_t[:], bq)
        xts = load_xin(xk, 0)
        for m in range(DG // P):
            proj_qk_m(wk_hs, xts, bk_t, KTs[m][0], m)
        wv_hs = load_w_halves(wv)
        nc.sync.dma_start(bvr_t[:], bvr)
        for kt in range(NKT):
            nc.sync.dma_start(Vgs[kt][:, :, DK], onescol[:, kt, :])
        xtsv = load_xin(xv, 0)
        for mt_l in range(4):
            for h2 in range(2):
                proj_v_part(wv_hs, xtsv, 0, mt_l, h2)
        wq_hs = load_w_halves(wq)
        nc.sync.dma_start(mask_t[:], masktri)
        QTcur = []
        for m in range(DG // P):
            qt_t = qt_pool.tile([P, 512], f32r, tag=f"QTm{m}", name=f"QTm{m}")
            QTcur.append(qt_t)
        xtsq = load_xin(xq, 0)
        for m in range(DG // P):
            proj_qk_m(wq_hs, xtsq, bq_t, QTcur[m], m)
        wos = []
        for nb in range(2):                   # [DG, D] -> halves [128,4,512]
            wh = wo_pool.tile([P, DG // P, 512], f32r, tag="wo", name="wo")
            nc.sync.dma_start(
                wh[:], wo[:, nb * 512:(nb + 1) * 512]
                .rearrange("(o p) m -> p o m", p=P))
            wos.append(wh)

        prevOT = None
        for qb in range(NQB):
            fillers = []
            if prevOT is not None:
                for qt_l in range(4):
                    for nb in range(2):
                        fillers.append(
                            lambda qt_l=qt_l, nb=nb, O=prevOT, q=qb - 1:
                            yproj_part(q, O, qt_l, nb))
            nextQT = None
            if qb + 1 < NQB:
                nextQT, pf = proj_block(qb + 1)
                fillers.extend(pf)
            prevOT = attention_qb(qb, QTcur, fillers)
            QTcur = nextQT
        for qt_l in range(4):
            for nb in range(2):
                yproj_part(NQB - 1, prevOT, qt_l, nb)

    nc.compile()
    return nc


def _prep_inputs(query, key, value, Wq, bq, Wk, bk, Wv, bv, Wo, bo):
    scale = 1.0 / np.sqrt(DK)
    qr = _to_f32r(np.asarray(query))
    kr = _to_f32r(np.asarray(key))
    vr = _to_f32r(np.asarray(value))
    wq_full = _to_f32r(np.asarray(Wq).T * scale)   # [D, D], cols = out chans
    wk_full = _to_f32r(np.asarray(Wk).T)
    wv_full = _to_f32r(np.asarray(Wv).T)
    wo_full = _to_f32r(np.asarray(Wo).T)           # [Din, Dout]
    bq_s = np.asarray(bq) * scale

    # constant tensors
    jj = np.arange(P)[:, None]
    ff = np.arange(P)[None, :]
    masktri = np.where(jj <= ff, 1.0, 0.0).astype(np.float32)
    onescol = np.ones((P, NKT, HG), np.float32)

    in_maps = []
    for core in range(8):
        b, hg = core // 2, core % 2
        sl = slice(hg * DG, (hg + 1) * DG)
        in_maps.append({
            "xq": np.ascontiguousarray(qr[b].T),
            "xk": np.ascontiguousarray(kr[b].T),
            "xv": np.ascontiguousarray(vr[b].T),
            "wq": np.ascontiguousarray(wq_full[:, sl]),
            "wk": np.ascontiguousarray(wk_full[:, sl]),
            "wv": np.ascontiguousarray(wv_full[:, sl]),
            "wo": np.ascontiguousarray(wo_full[sl, :]),
            "bq": np.ascontiguousarray(
                bq_s[sl].reshape(DG // P, P).T.astype(np.float32)),
            "bk": np.ascontiguousarray(
                np.asarray(bk)[sl].reshape(DG // P, P).T.astype(np.float32)),
            "bvr": np.broadcast_to(
                np.asarray(bv)[sl].astype(np.float32), (P, DG)).copy(),
            "masktri": masktri,
            "onescol": onescol,
        })
    return in_maps


def kernel(query, key, value, mask, Wq, bq, Wk, bk, Wv, bv, Wo, bo,
           **run_kwargs):
    from concourse.bass_utils import run_bass_kernel_spmd

    if "nc" not in _cache:
        _cache["nc"] = _build()
    nc = _cache["nc"]

    in_maps = _prep_inputs(query, key, value, Wq, bq, Wk, bk, Wv, bv, Wo, bo)
    res = run_bass_kernel_spmd(nc, in_maps, core_ids=list(range(8)),
                               **run_kwargs)
    bo = np.asarray(bo, dtype=np.float32)
    out = np.empty((B, S, D), dtype=np.float32)
    for b in range(B):
        out[b] = res.results[2 * b]["y"] + res.results[2 * b + 1]["y"] + bo
    _cache["last_results"] = res
    return out
